# revision 11
# baseline (speedup 1.0000x reference)
"""Trainium2 Bass kernel for pre-LN multi-head attention block.

Reference computation (per batch element):
  xn = LayerNorm(x) * gamma + beta                 [N, D]
  qkv = xn @ w_qkv.T                               [N, 3*INNER]
  q, k, v -> [H, N, Dh]; attn = softmax(q k^T / sqrt(Dh)); o = attn @ v
  out = o @ w_proj.T + b_proj                      [N, D]

Sharding: data-parallel over batch B=8 across the 8 NeuronCores (one batch
element per core, no collectives).

Shapes (hardcoded): B=8, N=2048, D=512, H=8, Dh=64, INNER=512.
"""

import os
import numpy as np
import ml_dtypes

import concourse.bass as bass
import concourse.mybir as mybir
import concourse.tile as tile
from concourse import bacc, masks

F32 = mybir.dt.float32
BF16 = mybir.dt.bfloat16

B = 8
N = 2048
D = 512
H = 8
Dh = 64
INNER = H * Dh  # 512
EPS = 1e-6
SCALE = Dh ** -0.5  # 0.125

P = 128
NT = N // P       # 16 token tiles
DC = D // P       # 4 d-chunks
QT = 4            # q tiles of 512
QW = N // QT      # 512
KC = N // P       # 16 key chunks of 128
HT = H // 2       # 4 head pairs (2 heads share a 128-partition tile)


def build_graph():
    nc = bacc.Bacc()

    x = nc.declare_dram_parameter("x", [N, D], F32, isOutput=False)
    w_qkvT = nc.declare_dram_parameter("w_qkvT", [D, 3 * INNER], BF16, isOutput=False)
    w_projT = nc.declare_dram_parameter("w_projT", [INNER, D], BF16, isOutput=False)
    b_proj = nc.declare_dram_parameter("b_proj", [D], F32, isOutput=False)
    gamma = nc.declare_dram_parameter("ln_gamma", [D], F32, isOutput=False)
    beta = nc.declare_dram_parameter("ln_beta", [D], F32, isOutput=False)
    out = nc.declare_dram_parameter("out", [N, D], F32, isOutput=True)

    def bcast_ap(ap_1d, parts):
        # DRAM [D] -> [parts, D] partition-broadcast access pattern
        return bass.AP(tensor=ap_1d.tensor, offset=ap_1d.offset,
                       ap=[[0, parts]] + list(ap_1d.ap))

    with tile.TileContext(nc) as tc:
        with (
            tc.tile_pool(name="consts", bufs=1) as consts,
            tc.tile_pool(name="big", bufs=1) as big,
            tc.tile_pool(name="ln", bufs=3) as ln,
            tc.tile_pool(name="xload", bufs=NT) as xload,
            tc.tile_pool(name="yout", bufs=NT) as yout,
            tc.tile_pool(name="work", bufs=3) as work,
            tc.tile_pool(name="small", bufs=4) as small,
            tc.tile_pool(name="mm_ps", bufs=2, space="PSUM") as mm_ps,
            tc.tile_pool(name="s_ps", bufs=2, space="PSUM") as s_ps,
            tc.tile_pool(name="o_ps", bufs=1, space="PSUM") as o_ps,
        ):
            # ---- constants ----
            wq = consts.tile([P, DC, 3 * INNER], BF16)
            nc.sync.dma_start(wq, w_qkvT.rearrange("(o p) f -> p o f", p=P))
            wp = consts.tile([P, DC, D], BF16)
            nc.sync.dma_start(wp, w_projT.rearrange("(o p) f -> p o f", p=P))
            bias_bc = consts.tile([P, D], F32)
            nc.sync.dma_start(bias_bc, bcast_ap(b_proj[:], P))
            gamma_bc = consts.tile([P, D], F32)
            nc.sync.dma_start(gamma_bc, bcast_ap(gamma[:], P))
            beta_bc = consts.tile([P, D], F32)
            nc.sync.dma_start(beta_bc, bcast_ap(beta[:], P))
            eps_t = consts.tile([P, 1], F32)
            nc.vector.memset(eps_t, EPS)
            ones1 = consts.tile([1, Dh], BF16)
            nc.vector.memset(ones1, 1.0)
            ident = consts.tile([P, P], BF16)
            masks.make_identity(nc, ident)

            # ---- LayerNorm + transpose: x [N, D] -> xnT [128, DC, N] bf16 ----
            xnT = big.tile([P, DC, N], BF16)
            for i in range(NT):
                xt = xload.tile([P, D], F32)
                nc.sync.dma_start(xt, x[i * P:(i + 1) * P, :])
                stats = ln.tile([P, 6], F32)
                nc.vector.bn_stats(stats, xt)
                mv = ln.tile([P, 2], F32)
                nc.vector.bn_aggr(mv, stats)
                std = ln.tile([P, 1], F32)
                nc.scalar.activation(std, mv[:, 1:2],
                                     mybir.ActivationFunctionType.Sqrt,
                                     bias=eps_t)
                rstd = ln.tile([P, 1], F32)
                nc.vector.reciprocal(rstd, std)
                xn_f = ln.tile([P, D], F32)
                nc.vector.tensor_scalar(out=xn_f, in0=xt,
                                        scalar1=mv[:, 0:1], scalar2=rstd,
                                        op0=mybir.AluOpType.subtract,
                                        op1=mybir.AluOpType.mult)
                nc.vector.tensor_tensor(xn_f, xn_f, gamma_bc, mybir.AluOpType.mult)
                nc.vector.tensor_tensor(xn_f, xn_f, beta_bc, mybir.AluOpType.add)
                xn_b = ln.tile([P, D], BF16)
                nc.vector.tensor_copy(xn_b, xn_f)
                for dc in range(DC):
                    pt = mm_ps.tile([P, P], BF16, tag="ps")
                    nc.tensor.transpose(pt, xn_b[:, dc * P:(dc + 1) * P], ident)
                    nc.vector.tensor_copy(xnT[:, dc, i * P:(i + 1) * P], pt)

            # ---- QKV projections ----
            # QT_t / KT_t: [128, HT, N] (feature rows on partitions; head pair
            # t has head 2t in rows 0:64 and head 2t+1 in rows 64:128)
            qT = big.tile([P, HT, N], BF16)
            kT = big.tile([P, HT, N], BF16)
            # V_aug: [128, KC, H, Dh+1] token chunks on partitions, per head a
            # trailing ones column (accumulates the softmax denominator)
            v_aug = big.tile([P, KC, H, Dh + 1], BF16)
            nc.vector.memset(v_aug[:, :, :, Dh:Dh + 1], 1.0)

            for f in range(2 * HT):  # 8 feature tiles of 128 (Q: 0-3, K: 4-7)
                dest = qT if f < HT else kT
                ft = f % HT
                for s in range(QT):
                    ps = mm_ps.tile([P, QW], F32)
                    for dc in range(DC):
                        nc.tensor.matmul(ps,
                                         lhsT=wq[:, dc, f * P:(f + 1) * P],
                                         rhs=xnT[:, dc, s * QW:(s + 1) * QW],
                                         start=(dc == 0), stop=(dc == DC - 1))
                    nc.vector.tensor_copy(dest[:, ft, s * QW:(s + 1) * QW], ps)

            for nt in range(NT):  # V branch: [n, feat] layout
                ps = mm_ps.tile([P, INNER], F32)
                for dc in range(DC):
                    nc.tensor.matmul(ps,
                                     lhsT=xnT[:, dc, nt * P:(nt + 1) * P],
                                     rhs=wq[:, dc, 2 * INNER:3 * INNER],
                                     start=(dc == 0), stop=(dc == DC - 1))
                nc.vector.tensor_copy(
                    v_aug[:, nt, :, 0:Dh],
                    ps[:, :].rearrange("p (h c) -> p h c", h=H))

            # ---- attention ----
            # oT: [128, DC, N] bf16 — INNER rows on partitions (head h at
            # partition (h%2)*64, free tile h//2), ready as proj lhsT chunks.
            oT = big.tile([P, DC, N], BF16)

            for t in range(HT):
                for s in range(QT):
                    po_a = o_ps.tile([P, QW], F32, tag="po_a")
                    po_b = o_ps.tile([P, QW], F32, tag="po_b")
                    for kc in range(KC):
                        ps = s_ps.tile([P, 2 * QW], F32)
                        # S^T[k, q] for head pair: head A rows 0:64, B rows 64:128
                        nc.tensor.matmul(
                            ps[:, 0:QW],
                            lhsT=kT[0:Dh, t, kc * P:(kc + 1) * P],
                            rhs=qT[0:Dh, t, s * QW:(s + 1) * QW],
                            start=True, stop=True)
                        nc.tensor.matmul(
                            ps[:, QW:2 * QW],
                            lhsT=kT[Dh:P, t, kc * P:(kc + 1) * P],
                            rhs=qT[Dh:P, t, s * QW:(s + 1) * QW],
                            start=True, stop=True)
                        et = work.tile([P, 2 * QW], BF16)
                        nc.scalar.activation(et, ps,
                                             mybir.ActivationFunctionType.Exp,
                                             scale=SCALE)
                        nc.tensor.matmul(po_a[0:Dh + 1, :],
                                         lhsT=v_aug[:, kc, 2 * t, :],
                                         rhs=et[:, 0:QW],
                                         start=(kc == 0), stop=(kc == KC - 1))
                        nc.tensor.matmul(po_b[0:Dh + 1, :],
                                         lhsT=v_aug[:, kc, 2 * t + 1, :],
                                         rhs=et[:, QW:2 * QW],
                                         start=(kc == 0), stop=(kc == KC - 1))
                    # normalize: O = O~ / rowsum (rowsum in row 64)
                    for h_off, po in ((0, po_a), (1, po_b)):
                        rs = small.tile([1, QW], F32, tag="rs")
                        nc.vector.reciprocal(rs, po[Dh:Dh + 1, :])
                        rsb = small.tile([1, QW], BF16, tag="rsb")
                        nc.vector.tensor_copy(rsb, rs)
                        pr = mm_ps.tile([P, QW], F32, tag="ps")
                        nc.tensor.matmul(pr[0:Dh, :], lhsT=ones1, rhs=rsb,
                                         start=True, stop=True)
                        ot_tmp = small.tile([Dh, QW], BF16, tag="ot_tmp")
                        nc.vector.tensor_copy(ot_tmp, po[0:Dh, :])
                        nc.vector.tensor_tensor(
                            oT[h_off * Dh:(h_off + 1) * Dh, t, s * QW:(s + 1) * QW],
                            ot_tmp, pr[0:Dh, :], mybir.AluOpType.mult)

            # ---- output projection: y [n, dout] = oT.T @ w_projT + b ----
            for nt in range(NT):
                ps = mm_ps.tile([P, D], F32)
                for c in range(DC):
                    nc.tensor.matmul(ps,
                                     lhsT=oT[:, c, nt * P:(nt + 1) * P],
                                     rhs=wp[:, c, :],
                                     start=(c == 0), stop=(c == DC - 1))
                yt = yout.tile([P, D], F32)
                nc.vector.tensor_tensor(yt, ps, bias_bc, mybir.AluOpType.add)
                nc.sync.dma_start(out[nt * P:(nt + 1) * P, :], yt)

    nc.compile()
    return nc


_CACHED = {}


def _prep_weights(w_qkv, w_proj, b_proj, ln_gamma, ln_beta):
    return {
        "w_qkvT": np.ascontiguousarray(w_qkv.T).astype(ml_dtypes.bfloat16),
        "w_projT": np.ascontiguousarray(w_proj.T).astype(ml_dtypes.bfloat16),
        "b_proj": np.ascontiguousarray(b_proj).astype(np.float32),
        "ln_gamma": np.ascontiguousarray(ln_gamma).astype(np.float32),
        "ln_beta": np.ascontiguousarray(ln_beta).astype(np.float32),
    }


def kernel(x, w_qkv, w_proj, b_proj, ln_gamma, ln_beta):
    from concourse.bass_utils import run_bass_kernel_spmd

    x = np.asarray(x, dtype=np.float32)
    assert x.shape == (B, N, D), x.shape

    if "nc" not in _CACHED:
        _CACHED["nc"] = build_graph()
    nc = _CACHED["nc"]

    shared = _prep_weights(np.asarray(w_qkv), np.asarray(w_proj),
                           np.asarray(b_proj), np.asarray(ln_gamma),
                           np.asarray(ln_beta))
    in_maps = [dict(shared, x=np.ascontiguousarray(x[i])) for i in range(B)]

    trace = bool(int(os.environ.get("KERNEL_TRACE", "0")))
    res = run_bass_kernel_spmd(nc, in_maps, core_ids=list(range(B)),
                               trace=trace)
    if trace:
        _CACHED["exec_time_ns"] = res.exec_time_ns
        _CACHED["last_result"] = res
    outs = [np.asarray(res.results[i]["out"], dtype=np.float32)
            for i in range(B)]
    return np.stack(outs, axis=0)


# revision 15
# speedup vs baseline: 1.3741x; 1.3741x over previous
"""Trainium2 Bass kernel for pre-LN multi-head attention block.

Reference computation (per batch element):
  xn = LayerNorm(x) * gamma + beta                 [N, D]
  qkv = xn @ w_qkv.T                               [N, 3*INNER]
  q, k, v -> [H, N, Dh]; attn = softmax(q k^T / sqrt(Dh)); o = attn @ v
  out = o @ w_proj.T + b_proj                      [N, D]

Sharding: data-parallel over batch B=8 across the 8 NeuronCores (one batch
element per core, no collectives).

Shapes (hardcoded): B=8, N=2048, D=512, H=8, Dh=64, INNER=512.
"""

import os
import numpy as np
import ml_dtypes

import concourse.bass as bass
import concourse.mybir as mybir
import concourse.tile as tile
from concourse import bacc, masks

F32 = mybir.dt.float32
BF16 = mybir.dt.bfloat16

B = 8
N = 2048
D = 512
H = 8
Dh = 64
INNER = H * Dh  # 512
EPS = 1e-6
SCALE = Dh ** -0.5  # 0.125

P = 128
NT = N // P       # 16 token tiles
DC = D // P       # 4 d-chunks
QT = 4            # q tiles of 512
QW = N // QT      # 512
KC = N // P       # 16 key chunks of 128
HT = H // 2       # 4 head pairs (2 heads share a 128-partition tile)


def build_graph():
    nc = bacc.Bacc()

    x = nc.declare_dram_parameter("x", [N, D], F32, isOutput=False)
    w_qkvT = nc.declare_dram_parameter("w_qkvT", [D, 3 * INNER], BF16, isOutput=False)
    w_projT = nc.declare_dram_parameter("w_projT", [INNER, D], BF16, isOutput=False)
    b_proj = nc.declare_dram_parameter("b_proj", [D], F32, isOutput=False)
    gamma = nc.declare_dram_parameter("ln_gamma", [D], F32, isOutput=False)
    beta = nc.declare_dram_parameter("ln_beta", [D], F32, isOutput=False)
    out = nc.declare_dram_parameter("out", [N, D], F32, isOutput=True)

    def bcast_ap(ap_1d, parts):
        # DRAM [D] -> [parts, D] partition-broadcast access pattern
        return bass.AP(tensor=ap_1d.tensor, offset=ap_1d.offset,
                       ap=[[0, parts]] + list(ap_1d.ap))

    with tile.TileContext(nc) as tc:
        with (
            tc.tile_pool(name="consts", bufs=1) as consts,
            tc.tile_pool(name="big", bufs=1) as big,
            tc.tile_pool(name="ln", bufs=3) as ln,
            tc.tile_pool(name="xload", bufs=4) as xload,
            tc.tile_pool(name="yout", bufs=4) as yout,
            tc.tile_pool(name="work", bufs=3) as work,
            tc.tile_pool(name="small", bufs=4) as small,
            tc.tile_pool(name="s_ps", bufs=2, space="PSUM") as s_ps,
            tc.tile_pool(name="o_ps", bufs=2, space="PSUM") as o_ps,
        ):
            # o_ps holds two [128, 512] f32 tags (po_a / po_b); the QKV /
            # proj / transpose phases borrow its slots (same bank budget).
            _mm_ctr = [0]

            def mm_ps_tile(shape, dtype):
                _mm_ctr[0] += 1
                tag = "po_a" if _mm_ctr[0] % 2 else "po_b"
                return o_ps.tile(shape, dtype, tag=tag,
                                 name=f"mm_{_mm_ctr[0]}")
            # ---- constants ----
            wq = consts.tile([P, DC, 3 * INNER], BF16)
            nc.sync.dma_start(wq, w_qkvT.rearrange("(o p) f -> p o f", p=P))
            wp = consts.tile([P, DC, D], BF16)
            nc.sync.dma_start(wp, w_projT.rearrange("(o p) f -> p o f", p=P))
            bias_bc = consts.tile([P, D], F32)
            nc.sync.dma_start(bias_bc, bcast_ap(b_proj[:], P))
            gamma_bc = consts.tile([P, D], F32)
            nc.sync.dma_start(gamma_bc, bcast_ap(gamma[:], P))
            beta_bc = consts.tile([P, D], F32)
            nc.sync.dma_start(beta_bc, bcast_ap(beta[:], P))
            eps_t = consts.tile([P, 1], F32)
            nc.vector.memset(eps_t, EPS)
            ident = consts.tile([P, P], BF16)
            masks.make_identity(nc, ident)

            # ---- LayerNorm + transpose: x [N, D] -> xnT [128, DC, N] bf16 ----
            xnT = big.tile([P, DC, N], BF16)
            for i in range(NT):
                xt = xload.tile([P, D], F32)
                nc.sync.dma_start(xt, x[i * P:(i + 1) * P, :])
                stats = ln.tile([P, 6], F32)
                nc.vector.bn_stats(stats, xt)
                mv = ln.tile([P, 2], F32)
                nc.vector.bn_aggr(mv, stats)
                std = ln.tile([P, 1], F32)
                nc.scalar.activation(std, mv[:, 1:2],
                                     mybir.ActivationFunctionType.Sqrt,
                                     bias=eps_t)
                rstd = ln.tile([P, 1], F32)
                nc.vector.reciprocal(rstd, std)
                xn_f = ln.tile([P, D], F32)
                nc.vector.tensor_scalar(out=xn_f, in0=xt,
                                        scalar1=mv[:, 0:1], scalar2=rstd,
                                        op0=mybir.AluOpType.subtract,
                                        op1=mybir.AluOpType.mult)
                nc.vector.tensor_tensor(xn_f, xn_f, gamma_bc, mybir.AluOpType.mult)
                nc.vector.tensor_tensor(xn_f, xn_f, beta_bc, mybir.AluOpType.add)
                xn_b = ln.tile([P, D], BF16)
                nc.vector.tensor_copy(xn_b, xn_f)
                for dc in range(DC):
                    pt = mm_ps_tile([P, P], BF16)
                    nc.tensor.transpose(pt, xn_b[:, dc * P:(dc + 1) * P], ident)
                    nc.vector.tensor_copy(xnT[:, dc, i * P:(i + 1) * P], pt)

            # ---- QKV projections ----
            # QT_t / KT_t: [128, HT, N] (feature rows on partitions; head pair
            # t has head 2t in rows 0:64 and head 2t+1 in rows 64:128)
            qT = big.tile([P, HT, N], BF16)
            kT = big.tile([P, HT, N], BF16)
            # V_aug: [128, KC, H, Dh+1] token chunks on partitions, per head a
            # trailing ones column (accumulates the softmax denominator)
            v_aug = big.tile([P, KC, H, Dh + 1], BF16)
            nc.vector.memset(v_aug[:, :, :, Dh:Dh + 1], 1.0)

            for f in range(2 * HT):  # 8 feature tiles of 128 (Q: 0-3, K: 4-7)
                dest = qT if f < HT else kT
                ft = f % HT
                for s in range(QT):
                    ps = mm_ps_tile([P, QW], F32)
                    for dc in range(DC):
                        nc.tensor.matmul(ps,
                                         lhsT=wq[:, dc, f * P:(f + 1) * P],
                                         rhs=xnT[:, dc, s * QW:(s + 1) * QW],
                                         start=(dc == 0), stop=(dc == DC - 1))
                    nc.vector.tensor_copy(dest[:, ft, s * QW:(s + 1) * QW], ps)

            for nt in range(NT):  # V branch: [n, feat] layout
                ps = mm_ps_tile([P, INNER], F32)
                for dc in range(DC):
                    nc.tensor.matmul(ps,
                                     lhsT=xnT[:, dc, nt * P:(nt + 1) * P],
                                     rhs=wq[:, dc, 2 * INNER:3 * INNER],
                                     start=(dc == 0), stop=(dc == DC - 1))
                nc.vector.tensor_copy(
                    v_aug[:, nt, :, 0:Dh],
                    ps[:, :].rearrange("p (h c) -> p h c", h=H))

            # ---- attention ----
            # oT: [128, DC, N] bf16 — INNER rows on partitions (head h at
            # partition (h%2)*64, free tile h//2), ready as proj lhsT chunks.
            oT = big.tile([P, DC, N], BF16)

            for t in range(HT):
                for s in range(QT):
                    po_a = o_ps.tile([P, QW], F32, tag="po_a")
                    po_b = o_ps.tile([P, QW], F32, tag="po_b")
                    for kc in range(KC):
                        ps = s_ps.tile([P, 2 * QW], F32)
                        # S^T[k, q] for head pair: head A rows 0:64, B rows 64:128
                        nc.tensor.matmul(
                            ps[:, 0:QW],
                            lhsT=kT[0:Dh, t, kc * P:(kc + 1) * P],
                            rhs=qT[0:Dh, t, s * QW:(s + 1) * QW],
                            start=True, stop=True)
                        nc.tensor.matmul(
                            ps[:, QW:2 * QW],
                            lhsT=kT[Dh:P, t, kc * P:(kc + 1) * P],
                            rhs=qT[Dh:P, t, s * QW:(s + 1) * QW],
                            start=True, stop=True)
                        et = work.tile([P, 2 * QW], BF16)
                        nc.scalar.activation(et, ps,
                                             mybir.ActivationFunctionType.Exp,
                                             scale=SCALE)
                        nc.tensor.matmul(po_a[0:Dh + 1, :],
                                         lhsT=v_aug[:, kc, 2 * t, :],
                                         rhs=et[:, 0:QW],
                                         start=(kc == 0), stop=(kc == KC - 1))
                        nc.tensor.matmul(po_b[0:Dh + 1, :],
                                         lhsT=v_aug[:, kc, 2 * t + 1, :],
                                         rhs=et[:, QW:2 * QW],
                                         start=(kc == 0), stop=(kc == KC - 1))
                    # normalize: O = O~ / rowsum (rowsum in row 64).
                    # Read PSUM out quickly (frees the bank for the next
                    # q-tile), then recip+broadcast off the critical path.
                    for h_off, po in ((0, po_a), (1, po_b)):
                        rs = small.tile([1, QW], F32, tag="rs")
                        nc.vector.tensor_copy(rs, po[Dh:Dh + 1, :])
                        ot_tmp = small.tile([Dh, QW], BF16, tag="ot_tmp")
                        nc.vector.tensor_copy(ot_tmp, po[0:Dh, :])
                        rr = small.tile([1, QW], F32, tag="rr")
                        nc.vector.reciprocal_approx_fast(out=rr, in_=rs)
                        rb = small.tile([Dh, QW], F32, tag="rb")
                        nc.gpsimd.partition_broadcast(rb, rr)
                        nc.vector.tensor_tensor(
                            oT[h_off * Dh:(h_off + 1) * Dh, t, s * QW:(s + 1) * QW],
                            ot_tmp, rb, mybir.AluOpType.mult)

            # ---- output projection: y [n, dout] = oT.T @ w_projT + b ----
            for nt in range(NT):
                ps = mm_ps_tile([P, D], F32)
                for c in range(DC):
                    nc.tensor.matmul(ps,
                                     lhsT=oT[:, c, nt * P:(nt + 1) * P],
                                     rhs=wp[:, c, :],
                                     start=(c == 0), stop=(c == DC - 1))
                yt = yout.tile([P, D], F32)
                nc.vector.tensor_tensor(yt, ps, bias_bc, mybir.AluOpType.add)
                nc.sync.dma_start(out[nt * P:(nt + 1) * P, :], yt)

    nc.compile()
    return nc


_CACHED = {}


def _prep_weights(w_qkv, w_proj, b_proj, ln_gamma, ln_beta):
    return {
        "w_qkvT": np.ascontiguousarray(w_qkv.T).astype(ml_dtypes.bfloat16),
        "w_projT": np.ascontiguousarray(w_proj.T).astype(ml_dtypes.bfloat16),
        "b_proj": np.ascontiguousarray(b_proj).astype(np.float32),
        "ln_gamma": np.ascontiguousarray(ln_gamma).astype(np.float32),
        "ln_beta": np.ascontiguousarray(ln_beta).astype(np.float32),
    }


def kernel(x, w_qkv, w_proj, b_proj, ln_gamma, ln_beta):
    from concourse.bass_utils import run_bass_kernel_spmd

    x = np.asarray(x, dtype=np.float32)
    assert x.shape == (B, N, D), x.shape

    if "nc" not in _CACHED:
        _CACHED["nc"] = build_graph()
    nc = _CACHED["nc"]

    shared = _prep_weights(np.asarray(w_qkv), np.asarray(w_proj),
                           np.asarray(b_proj), np.asarray(ln_gamma),
                           np.asarray(ln_beta))
    in_maps = [dict(shared, x=np.ascontiguousarray(x[i])) for i in range(B)]

    trace = bool(int(os.environ.get("KERNEL_TRACE", "0")))
    res = run_bass_kernel_spmd(nc, in_maps, core_ids=list(range(B)),
                               trace=trace)
    if trace:
        _CACHED["exec_time_ns"] = res.exec_time_ns
        _CACHED["last_result"] = res
    outs = [np.asarray(res.results[i]["out"], dtype=np.float32)
            for i in range(B)]
    return np.stack(outs, axis=0)
